# revision 56
# baseline (speedup 1.0000x reference)
"""Causal single-head attention on 8 trn2 NeuronCores.

Problem: x[4,2048,1024], Wq/Wk/Wv[1024,64] ->
  softmax(causal((x@Wq)@(x@Wk).T / 32)) @ (x@Wv)  -> [4,2048,64]

Sharding: 8 cores = 4 batches x 2 query-shards. Zigzag query split for
causal load balance: shard A handles query blocks {0,3} (of 512 rows),
shard B handles {1,2}. Each core redundantly computes K/V for the key
blocks it needs from a host-transposed x[b].T.

SPMD uniformity: one program for all 8 cores. Per-core differences are
absorbed into data:
  - xt column-block permutation (A: [0,1,3,2], B: [1,0,2,3]) puts each
    core's diagonal (q==k) blocks at fixed program slots: q blocks live
    at xt slots 0 and 2, so pair (qslot I, kslot 0) and (II, kslot 2)
    are always the diagonal pairs; their triangular strip masks are a
    compile-time triu pattern.
  - a tiny per-core bias input kills fully-masked (dummy) pairs via the
    exp() bias: exp(s - 1e5) == 0.

Data path is bf16 (x and weights cast on host; k/v/q tiles, exp tiles
and v-augmented tiles bf16) with all matmul accumulation in f32 PSUM.
The bf16 PE path is ~2x the fp32r rate and halves the x DMA bytes.

On-chip layout (scores kept transposed so softmax denominators and the
attention*V product are plain matmuls):
  qT[64,1024], kvT[128,2048] = (Wq|Wk|Wv)^T @ xt   (bf16 matmuls)
  scoresT[k,q] = kT_tile.T @ qT                     (per 128k x 512q tile)
  exp via ScalarE with per-pair bias; diag strips masked by triu mult
  out_augT[65,512q] += v_aug_tile.T @ expT  where v_aug = [v | 1] gives
    the softmax denominator for free in row 64
  finalize: PE-transpose out_augT, divide rows by denominator, DMA out.
"""

import os
import sys

import numpy as np

if "/opt/trn_rl_repo" not in sys.path and os.path.isdir("/opt/trn_rl_repo"):
    sys.path.insert(0, "/opt/trn_rl_repo")

import ml_dtypes

import concourse.bacc as bacc
import concourse.mybir as mybir
import concourse.tile as tile
from concourse.bass_utils import run_bass_kernel_spmd

B, S, E, H = 4, 2048, 1024, 64
BLK = 512  # kv/q block (4 blocks per sequence)
NCORES = 8
NE = E // 128  # 8 e-tiles
F32 = mybir.dt.float32
F32R = mybir.dt.float32r
BF16 = mybir.dt.bfloat16
FEXP = mybir.ActivationFunctionType.Exp

# per-shard: query blocks and xt column-block permutation
QBLOCKS = {0: (0, 3), 1: (1, 2)}
PERM = {0: (0, 1, 3, 2), 1: (1, 0, 2, 3)}
# program-fixed pair list: (qslot, kslot); pairs 0 and 4 are diagonal
PAIRS = ((0, 0), (0, 1), (1, 0), (1, 1), (1, 2), (1, 3))
DIAG = (0, 4)
NEG = -1.0e5


def _build():
    nc = bacc.Bacc("TRN2", target_bir_lowering=False, debug=False, num_devices=NCORES)

    # xt host-packed as [128, (slot chunk etile col)] so every DMA descriptor
    # covers a 4KB contiguous run per partition (2 chunks of 4 e-tiles/slot)
    xt = nc.dram_tensor("xt", [128, NE * S], BF16, kind="ExternalInput").ap()
    # weights host-prearranged to SBUF layout: [p, (e h)], wkv then wq
    wqkv = nc.dram_tensor("wqkv", [128, NE * 3 * H], BF16, kind="ExternalInput").ap()
    bias2 = nc.dram_tensor("bias2", [128, 8], F32, kind="ExternalInput").ap()
    triu = nc.dram_tensor("triu", [128, 128], BF16, kind="ExternalInput").ap()
    ones2 = nc.dram_tensor("ones2", [128, 32], BF16, kind="ExternalInput").ap()
    idmat = nc.dram_tensor("idmat", [128, 128], F32, kind="ExternalInput").ap()
    idmatb = nc.dram_tensor("idmatb", [128, 128], BF16, kind="ExternalInput").ap()
    # out packed [128, (qs t h)]: col qs*256 + t*64 + c holds output row
    # qs*512 + t*128 + p  (1KB-contiguous DMA rows instead of 256B)
    out = nc.dram_tensor("out", [128, 8 * H], F32, kind="ExternalOutput").ap()

    with tile.TileContext(nc) as tc:
        with (
            tc.tile_pool(name="const", bufs=1) as cpool,
            tc.tile_pool(name="xt", bufs=3) as xtpool,
            tc.tile_pool(name="exp", bufs=4) as expool,
            tc.tile_pool(name="fin", bufs=2) as finpool,
            tc.tile_pool(name="kvps", bufs=1, space="PSUM") as kvps_pool,
            tc.tile_pool(name="qps", bufs=1, space="PSUM") as qps_pool,
            tc.tile_pool(name="vtps", bufs=1, space="PSUM") as vtps_pool,
            tc.tile_pool(name="stps", bufs=3, space="PSUM") as stps_pool,
            tc.tile_pool(name="avps", bufs=2, space="PSUM") as avps_pool,
        ):
            # ---- constants ----
            # weights in 4 chunk tiles so the first matmul only waits for
            # the e0-3 slice; xt rides the sync DGE ring alone, weights and
            # small constants ride the gpsimd ring
            wkv0_sb = cpool.tile([128, 2 * H], BF16)
            wkv123_sb = cpool.tile([128, 3 * 2 * H], BF16)
            wkv47_sb = cpool.tile([128, 4 * 2 * H], BF16)
            wq03_sb = cpool.tile([128, 4 * H], BF16)
            wq47_sb = cpool.tile([128, 4 * H], BF16)
            # only the e0 weight tile (32KB) leads the sync ring ahead of
            # xt, so the first matmul is gated by just 160KB; the rest of
            # the weights lead the gpsimd ring, then the small constants
            nc.sync.dma_start(out=wkv0_sb, in_=wqkv[:, 0 : 2 * H])
            nc.gpsimd.dma_start(out=wkv123_sb, in_=wqkv[:, 2 * H : 4 * 2 * H])
            nc.gpsimd.dma_start(out=wq03_sb, in_=wqkv[:, NE * 2 * H : NE * 2 * H + 4 * H])
            nc.gpsimd.dma_start(out=wkv47_sb, in_=wqkv[:, 4 * 2 * H : NE * 2 * H])
            nc.gpsimd.dma_start(out=wq47_sb, in_=wqkv[:, NE * 2 * H + 4 * H :])

            def wkv_e(e):
                if e == 0:
                    return wkv0_sb
                if e < 4:
                    return wkv123_sb[:, (e - 1) * 128 : e * 128]
                return wkv47_sb[:, (e % 4) * 128 : (e % 4 + 1) * 128]

            def wq_e(e):
                t = wq03_sb if e < 4 else wq47_sb
                return t[:, (e % 4) * H : (e % 4 + 1) * H]
            bias2_sb = cpool.tile([128, 8], F32)
            nc.gpsimd.dma_start(out=bias2_sb, in_=bias2)
            triu_sb = cpool.tile([128, 128], BF16)
            nc.gpsimd.dma_start(out=triu_sb, in_=triu)
            idmat_sb = cpool.tile([128, 128], F32R)
            nc.gpsimd.dma_start(out=idmat_sb, in_=idmat.bitcast(F32R))
            idmatb_sb = cpool.tile([128, 128], BF16)
            nc.gpsimd.dma_start(out=idmatb_sb, in_=idmatb)

            # persistent buffers
            kvT_sb = cpool.tile([128, S], BF16)  # rows 0:64 kT, 64:128 vT
            qT_sb = cpool.tile([64, 2 * BLK], BF16)
            # [v(64) | 1 | 1] per k-tile; 66-wide so the stationary free
            # count is even (col 65 is a harmless denominator dup)
            VA = H + 2
            vaug_sb = cpool.tile([128, 16 * VA], BF16)
            ones_sb = cpool.tile([128, 32], BF16)
            nc.gpsimd.dma_start(out=ones_sb, in_=ones2)
            nc.gpsimd.tensor_copy(
                vaug_sb.rearrange("p (t c) -> p t c", t=16)[:, :, H : H + 2],
                ones_sb.rearrange("p (t c) -> p t c", c=2),
            )

            av_ps = [None, None]  # group accumulators, created lazily

            # xt is packed [p, (slot e col)]: any e-range of a slot is
            # contiguous per partition. Slot 0 uses a small first chunk so
            # the first matmul starts as early as possible.
            CHUNKS = {0: (1, 3, 4), 1: (4, 4), 2: (4, 4), 3: (4, 4)}

            def proj_slot(s):
                """project xt column-block s -> kvT_sb[:, s*BLK:], qT if q slot,
                and v-natural tiles into vaug."""
                xts = []
                e0 = 0
                for ci, ne in enumerate(CHUNKS[s]):
                    xc = xtpool.tile(
                        [128, ne * BLK], BF16, name=f"xt_{s}_{ci}", tag=f"xt{ci}"
                    )
                    nc.sync.dma_start(
                        out=xc,
                        in_=xt[:, (s * NE + e0) * BLK : (s * NE + e0 + ne) * BLK],
                    )
                    xts += [xc[:, i * BLK : (i + 1) * BLK] for i in range(ne)]
                    e0 += ne
                kv_ps = kvps_pool.tile([128, BLK], F32, name=f"kvps_{s}", tag="kv")
                q_ps = None
                if s in (0, 2):
                    q_ps = qps_pool.tile([64, BLK], F32, name=f"qps_{s}", tag="q")
                for e in range(NE):
                    nc.tensor.matmul(
                        kv_ps,
                        wkv_e(e),
                        xts[e],
                        start=(e == 0),
                        stop=(e == NE - 1),
                    )
                    if q_ps is not None:
                        nc.tensor.matmul(
                            q_ps,
                            wq_e(e),
                            xts[e],
                            start=(e == 0),
                            stop=(e == NE - 1),
                        )
                nc.vector.tensor_copy(kvT_sb[:, s * BLK : (s + 1) * BLK], kv_ps)
                if q_ps is not None:
                    qs = s // 2
                    nc.vector.tensor_copy(qT_sb[:, qs * BLK : (qs + 1) * BLK], q_ps)
                # v natural tiles for the AV product
                for j in range(4):
                    t = s * 4 + j
                    vt_ps = vtps_pool.tile([128, H + 2], BF16, name=f"vt_{t}", tag="vt")
                    nc.tensor.transpose(
                        vt_ps[:, 0:H],
                        kvT_sb[64:128, t * 128 : (t + 1) * 128],
                        idmatb_sb[64:128, 64:128],
                    )
                    nc.vector.tensor_copy(
                        vaug_sb[:, t * VA : t * VA + H], vt_ps[:, 0:H]
                    )

            def do_strip(p, j):
                qslot, kslot = PAIRS[p]
                diag = p in DIAG
                first = p in (0, 2)
                last = p in (1, 4)  # final AV emitted per av group (j==3)
                if True:
                    # for diagonal pairs, columns q < j*128 are fully masked:
                    # skip them in the scores matmul, the exp, and the AV
                    # accumulation (they are never read)
                    c0 = j * 128 if diag else 0
                    st_ps = stps_pool.tile([128, BLK], F32, name=f"st_{p}_{j}", tag="st")
                    nc.tensor.matmul(
                        st_ps[:, c0:],
                        kvT_sb[0:64, kslot * BLK + j * 128 : kslot * BLK + (j + 1) * 128],
                        qT_sb[0:64, qslot * BLK + c0 : (qslot + 1) * BLK],
                        start=True,
                        stop=True,
                    )
                    ex = expool.tile([128, BLK], BF16, name=f"ex_{p}_{j}", tag="ex")
                    if diag:
                        nc.scalar.activation(
                            ex[:, c0:], st_ps[:, c0:], FEXP, bias=0.0
                        )
                        nc.gpsimd.tensor_tensor(
                            ex[:, j * 128 : (j + 1) * 128],
                            ex[:, j * 128 : (j + 1) * 128],
                            triu_sb,
                            mybir.AluOpType.mult,
                        )
                    else:
                        nc.scalar.activation(
                            ex, st_ps, FEXP, bias=bias2_sb[:, p : p + 1]
                        )
                    if av_ps[qslot] is None:
                        av_ps[qslot] = avps_pool.tile(
                            [H + 2, BLK], F32, name=f"av_{qslot}", tag="av"
                        )
                    t = kslot * 4 + j
                    nc.tensor.matmul(
                        av_ps[qslot][:, c0:],
                        vaug_sb[:, t * VA : (t + 1) * VA],
                        ex[:, c0:],
                        start=(first and j == 0),
                        stop=(last and j == 3),
                        skip_group_check=True,
                    )

            def finalize(qs):
                oT_sb = finpool.tile([H + 2, BLK], F32R, name=f"oT_{qs}", tag="oT")
                nc.vector.tensor_copy(oT_sb, av_ps[qs])
                o4_sb = finpool.tile([128, 4 * H], F32, name=f"o4_{qs}", tag="o4")
                for t in range(4):
                    # qs=1 is the kernel tail: double-buffer its transposes
                    # across the vt and (idle by now) q PSUM slots so each
                    # iteration's vector reads overlap the next transpose
                    if qs == 1 and t % 2 == 1:
                        tr_ps = qps_pool.tile(
                            [128, H + 2], F32R, name=f"tr_{qs}_{t}", tag="q"
                        )
                    else:
                        tr_ps = vtps_pool.tile(
                            [128, H + 2], F32R, name=f"tr_{qs}_{t}", tag="vt"
                        )
                    nc.tensor.transpose(
                        tr_ps,
                        oT_sb[:, t * 128 : (t + 1) * 128],
                        idmat_sb[0 : H + 2, 0 : H + 2],
                    )
                    rden = finpool.tile([128, 1], F32, name=f"rd_{qs}_{t}", tag="rd")
                    nc.vector.reciprocal(rden, tr_ps[:, H : H + 1].bitcast(F32))
                    nc.vector.tensor_scalar_mul(
                        o4_sb[:, t * H : (t + 1) * H], tr_ps[:, 0:H].bitcast(F32), rden
                    )
                nc.sync.dma_start(
                    out=out[:, qs * 4 * H : (qs + 1) * 4 * H], in_=o4_sb
                )

            def do_pair(p):
                for j in range(4):
                    do_strip(p, j)

            proj_slot(0)
            do_pair(0)
            proj_slot(1)
            do_pair(1)
            finalize(0)
            proj_slot(2)
            do_pair(2)
            do_pair(3)  # needs only kvT slot1 + qT slot1: run mid-kernel
            proj_slot(3)
            # interleave the last two pairs at strip granularity so the
            # PE always has a matmul ready while the other strip's exp
            # runs; pair 4 is diagonal (narrowed exp) so the tail is
            # lighter on ScalarE than the 3-pair variant
            for j in range(4):
                if j == 3:
                    # last round: put the short diagonal exp (128 cols)
                    # after the full-width one so the final ACT->AV link
                    # on the critical path is the cheap one
                    do_strip(5, j)
                    do_strip(4, j)
                else:
                    do_strip(4, j)
                    do_strip(5, j)
            finalize(1)

    nc.compile()
    return nc


_NC_CACHE = None
RUN_KWARGS = {}  # test harness may set {"trace": True}
LAST_RESULTS = None  # BassKernelResults of the most recent run


def kernel(x, Wq, Wk, Wv):
    global _NC_CACHE, LAST_RESULTS
    x = np.asarray(x, dtype=np.float32)
    Wq = np.asarray(Wq, dtype=np.float32)
    Wk = np.asarray(Wk, dtype=np.float32)
    Wv = np.asarray(Wv, dtype=np.float32)
    bf = ml_dtypes.bfloat16

    def to_sb(w):  # [E, h] -> [128, NE*h] with e-tiles side by side
        h = w.shape[1]
        return np.ascontiguousarray(
            w.reshape(NE, 128, h).transpose(1, 0, 2).reshape(128, NE * h)
        )

    wq_s = to_sb(Wq / np.float32(E**0.5))
    wkv = to_sb(np.concatenate([Wk, Wv], axis=1))
    wqkv = np.ascontiguousarray(np.concatenate([wkv, wq_s], axis=1)).astype(bf)
    triu = np.triu(np.ones((128, 128), dtype=np.float32)).astype(bf)
    ones2 = np.ones((128, 32), dtype=bf)
    idmat = np.eye(128, dtype=np.float32)
    idmatb = idmat.astype(bf)

    in_maps = []
    for core in range(NCORES):
        b, shard = divmod(core, 2)
        perm = PERM[shard]
        xtf = x[b].T.astype(bf)  # [E, S]
        xt2 = np.concatenate([xtf[:, p * BLK : (p + 1) * BLK] for p in perm], axis=1)
        # pack to [128, slot(4) x etile(8) x col(512)]: any e-range of a
        # slot is contiguous per partition (>=1KB DMA descriptors)
        xt = np.ascontiguousarray(
            xt2.reshape(NE, 128, 4, BLK)
            .transpose(1, 2, 0, 3)
            .reshape(128, NE * S)
        )
        qb = QBLOCKS[shard]
        bias2 = np.zeros((128, 8), dtype=np.float32)
        bias2[:, 6] = NEG
        for p, (qslot, kslot) in enumerate(PAIRS):
            if perm[kslot] > qb[qslot]:  # key block entirely in the future
                bias2[:, p] = NEG
        in_maps.append(
            dict(
                xt=xt,
                wqkv=wqkv,
                bias2=bias2,
                triu=triu,
                idmat=idmat,
                idmatb=idmatb,
                ones2=ones2,
            )
        )

    if _NC_CACHE is None:
        _NC_CACHE = _build()
    res = run_bass_kernel_spmd(
        _NC_CACHE, in_maps, core_ids=list(range(NCORES)), **RUN_KWARGS
    )
    LAST_RESULTS = res

    out = np.empty((B, S, H), dtype=np.float32)
    for core in range(NCORES):
        b, shard = divmod(core, 2)
        # unpack [128, (qs t h)] -> rows qs*512 + t*128 + p
        o = (
            res.results[core]["out"]
            .reshape(128, 8, H)
            .transpose(1, 0, 2)
            .reshape(2, BLK, H)
        )
        for qs, blk in enumerate(QBLOCKS[shard]):
            out[b, blk * BLK : (blk + 1) * BLK, :] = o[qs]
    return out


# revision 58
# speedup vs baseline: 1.0021x; 1.0021x over previous
"""Causal single-head attention on 8 trn2 NeuronCores.

Problem: x[4,2048,1024], Wq/Wk/Wv[1024,64] ->
  softmax(causal((x@Wq)@(x@Wk).T / 32)) @ (x@Wv)  -> [4,2048,64]

Sharding: 8 cores = 4 batches x 2 query-shards. Zigzag query split for
causal load balance: shard A handles query blocks {0,3} (of 512 rows),
shard B handles {1,2}. Each core redundantly computes K/V for the key
blocks it needs from a host-transposed x[b].T.

SPMD uniformity: one program for all 8 cores. Per-core differences are
absorbed into data:
  - xt column-block permutation (A: [0,1,3,2], B: [1,0,2,3]) puts each
    core's diagonal (q==k) blocks at fixed program slots: q blocks live
    at xt slots 0 and 2, so pair (qslot I, kslot 0) and (II, kslot 2)
    are always the diagonal pairs; their triangular strip masks are a
    compile-time triu pattern.
  - a tiny per-core bias input kills fully-masked (dummy) pairs via the
    exp() bias: exp(s - 1e5) == 0.

Data path is bf16 (x and weights cast on host; k/v/q tiles, exp tiles
and v-augmented tiles bf16) with all matmul accumulation in f32 PSUM.
The bf16 PE path is ~2x the fp32r rate and halves the x DMA bytes.

On-chip layout (scores kept transposed so softmax denominators and the
attention*V product are plain matmuls):
  qT[64,1024], kvT[128,2048] = (Wq|Wk|Wv)^T @ xt   (bf16 matmuls)
  scoresT[k,q] = kT_tile.T @ qT                     (per 128k x 512q tile)
  exp via ScalarE with per-pair bias; diag strips masked by triu mult
  out_augT[65,512q] += v_aug_tile.T @ expT  where v_aug = [v | 1] gives
    the softmax denominator for free in row 64
  finalize: PE-transpose out_augT, divide rows by denominator, DMA out.
"""

import os
import sys

import numpy as np

if "/opt/trn_rl_repo" not in sys.path and os.path.isdir("/opt/trn_rl_repo"):
    sys.path.insert(0, "/opt/trn_rl_repo")

import ml_dtypes

import concourse.bacc as bacc
import concourse.mybir as mybir
import concourse.tile as tile
from concourse.bass_utils import run_bass_kernel_spmd

B, S, E, H = 4, 2048, 1024, 64
BLK = 512  # kv/q block (4 blocks per sequence)
NCORES = 8
NE = E // 128  # 8 e-tiles
F32 = mybir.dt.float32
F32R = mybir.dt.float32r
BF16 = mybir.dt.bfloat16
FEXP = mybir.ActivationFunctionType.Exp

# per-shard: query blocks and xt column-block permutation
QBLOCKS = {0: (0, 3), 1: (1, 2)}
PERM = {0: (0, 1, 3, 2), 1: (1, 0, 2, 3)}
# program-fixed pair list: (qslot, kslot); pairs 0 and 4 are diagonal
PAIRS = ((0, 0), (0, 1), (1, 0), (1, 1), (1, 2), (1, 3))
DIAG = (0, 4)
NEG = -1.0e5


def _build():
    nc = bacc.Bacc("TRN2", target_bir_lowering=False, debug=False, num_devices=NCORES)

    # xt host-packed as [128, (slot chunk etile col)] so every DMA descriptor
    # covers a 4KB contiguous run per partition (2 chunks of 4 e-tiles/slot)
    xt = nc.dram_tensor("xt", [128, NE * S], BF16, kind="ExternalInput").ap()
    # weights host-prearranged to SBUF layout: [p, (e h)], wkv then wq
    wqkv = nc.dram_tensor("wqkv", [128, NE * 3 * H], BF16, kind="ExternalInput").ap()
    bias2 = nc.dram_tensor("bias2", [128, 8], F32, kind="ExternalInput").ap()
    triu = nc.dram_tensor("triu", [128, 128], BF16, kind="ExternalInput").ap()
    ones2 = nc.dram_tensor("ones2", [128, 32], BF16, kind="ExternalInput").ap()
    idmat = nc.dram_tensor("idmat", [128, 128], F32, kind="ExternalInput").ap()
    idmatb = nc.dram_tensor("idmatb", [128, 128], BF16, kind="ExternalInput").ap()
    # out packed [128, (qs t h)]: col qs*256 + t*64 + c holds output row
    # qs*512 + t*128 + p  (1KB-contiguous DMA rows instead of 256B)
    out = nc.dram_tensor("out", [128, 8 * H], F32, kind="ExternalOutput").ap()

    with tile.TileContext(nc) as tc:
        with (
            tc.tile_pool(name="const", bufs=1) as cpool,
            tc.tile_pool(name="xt", bufs=4) as xtpool,
            tc.tile_pool(name="exp", bufs=6) as expool,
            tc.tile_pool(name="fin", bufs=2) as finpool,
            tc.tile_pool(name="kvps", bufs=1, space="PSUM") as kvps_pool,
            tc.tile_pool(name="qps", bufs=1, space="PSUM") as qps_pool,
            tc.tile_pool(name="vtps", bufs=1, space="PSUM") as vtps_pool,
            tc.tile_pool(name="stps", bufs=3, space="PSUM") as stps_pool,
            tc.tile_pool(name="avps", bufs=2, space="PSUM") as avps_pool,
        ):
            # ---- constants ----
            # weights in 4 chunk tiles so the first matmul only waits for
            # the e0-3 slice; xt rides the sync DGE ring alone, weights and
            # small constants ride the gpsimd ring
            wkv0_sb = cpool.tile([128, 2 * H], BF16)
            wkv123_sb = cpool.tile([128, 3 * 2 * H], BF16)
            wkv47_sb = cpool.tile([128, 4 * 2 * H], BF16)
            wq03_sb = cpool.tile([128, 4 * H], BF16)
            wq47_sb = cpool.tile([128, 4 * H], BF16)
            # only the e0 weight tile (32KB) leads the sync ring ahead of
            # xt, so the first matmul is gated by just 160KB; the rest of
            # the weights lead the gpsimd ring, then the small constants
            nc.sync.dma_start(out=wkv0_sb, in_=wqkv[:, 0 : 2 * H])
            nc.gpsimd.dma_start(out=wkv123_sb, in_=wqkv[:, 2 * H : 4 * 2 * H])
            nc.gpsimd.dma_start(out=wq03_sb, in_=wqkv[:, NE * 2 * H : NE * 2 * H + 4 * H])
            nc.gpsimd.dma_start(out=wkv47_sb, in_=wqkv[:, 4 * 2 * H : NE * 2 * H])
            nc.gpsimd.dma_start(out=wq47_sb, in_=wqkv[:, NE * 2 * H + 4 * H :])

            def wkv_e(e):
                if e == 0:
                    return wkv0_sb
                if e < 4:
                    return wkv123_sb[:, (e - 1) * 128 : e * 128]
                return wkv47_sb[:, (e % 4) * 128 : (e % 4 + 1) * 128]

            def wq_e(e):
                t = wq03_sb if e < 4 else wq47_sb
                return t[:, (e % 4) * H : (e % 4 + 1) * H]
            bias2_sb = cpool.tile([128, 8], F32)
            nc.gpsimd.dma_start(out=bias2_sb, in_=bias2)
            triu_sb = cpool.tile([128, 128], BF16)
            nc.gpsimd.dma_start(out=triu_sb, in_=triu)
            idmat_sb = cpool.tile([128, 128], F32R)
            nc.gpsimd.dma_start(out=idmat_sb, in_=idmat.bitcast(F32R))
            idmatb_sb = cpool.tile([128, 128], BF16)
            nc.gpsimd.dma_start(out=idmatb_sb, in_=idmatb)

            # persistent buffers
            kvT_sb = cpool.tile([128, S], BF16)  # rows 0:64 kT, 64:128 vT
            qT_sb = cpool.tile([64, 2 * BLK], BF16)
            # [v(64) | 1 | 1] per k-tile; 66-wide so the stationary free
            # count is even (col 65 is a harmless denominator dup)
            VA = H + 2
            vaug_sb = cpool.tile([128, 16 * VA], BF16)
            ones_sb = cpool.tile([128, 32], BF16)
            nc.gpsimd.dma_start(out=ones_sb, in_=ones2)
            nc.gpsimd.tensor_copy(
                vaug_sb.rearrange("p (t c) -> p t c", t=16)[:, :, H : H + 2],
                ones_sb.rearrange("p (t c) -> p t c", c=2),
            )

            av_ps = [None, None]  # group accumulators, created lazily

            # xt is packed [p, (slot e col)]: any e-range of a slot is
            # contiguous per partition. Slot 0 uses a small first chunk so
            # the first matmul starts as early as possible.
            CHUNKS = {0: (1, 3, 4), 1: (4, 4), 2: (4, 4), 3: (4, 4)}

            def proj_slot(s):
                """project xt column-block s -> kvT_sb[:, s*BLK:], qT if q slot,
                and v-natural tiles into vaug."""
                xts = []
                e0 = 0
                for ci, ne in enumerate(CHUNKS[s]):
                    xc = xtpool.tile(
                        [128, ne * BLK], BF16, name=f"xt_{s}_{ci}", tag=f"xt{ci}"
                    )
                    nc.sync.dma_start(
                        out=xc,
                        in_=xt[:, (s * NE + e0) * BLK : (s * NE + e0 + ne) * BLK],
                    )
                    xts += [xc[:, i * BLK : (i + 1) * BLK] for i in range(ne)]
                    e0 += ne
                kv_ps = kvps_pool.tile([128, BLK], F32, name=f"kvps_{s}", tag="kv")
                q_ps = None
                if s in (0, 2):
                    q_ps = qps_pool.tile([64, BLK], F32, name=f"qps_{s}", tag="q")
                for e in range(NE):
                    nc.tensor.matmul(
                        kv_ps,
                        wkv_e(e),
                        xts[e],
                        start=(e == 0),
                        stop=(e == NE - 1),
                    )
                    if q_ps is not None:
                        nc.tensor.matmul(
                            q_ps,
                            wq_e(e),
                            xts[e],
                            start=(e == 0),
                            stop=(e == NE - 1),
                        )
                nc.vector.tensor_copy(kvT_sb[:, s * BLK : (s + 1) * BLK], kv_ps)
                if q_ps is not None:
                    qs = s // 2
                    nc.vector.tensor_copy(qT_sb[:, qs * BLK : (qs + 1) * BLK], q_ps)
                # v natural tiles for the AV product
                for j in range(4):
                    t = s * 4 + j
                    vt_ps = vtps_pool.tile([128, H + 2], BF16, name=f"vt_{t}", tag="vt")
                    nc.tensor.transpose(
                        vt_ps[:, 0:H],
                        kvT_sb[64:128, t * 128 : (t + 1) * 128],
                        idmatb_sb[64:128, 64:128],
                    )
                    nc.vector.tensor_copy(
                        vaug_sb[:, t * VA : t * VA + H], vt_ps[:, 0:H]
                    )

            def do_strip(p, j):
                qslot, kslot = PAIRS[p]
                diag = p in DIAG
                first = p in (0, 2)
                last = p in (1, 5)
                if True:
                    # for diagonal pairs, columns q < j*128 are fully masked:
                    # skip them in the scores matmul, the exp, and the AV
                    # accumulation (they are never read)
                    c0 = j * 128 if diag else 0
                    st_ps = stps_pool.tile([128, BLK], F32, name=f"st_{p}_{j}", tag="st")
                    nc.tensor.matmul(
                        st_ps[:, c0:],
                        kvT_sb[0:64, kslot * BLK + j * 128 : kslot * BLK + (j + 1) * 128],
                        qT_sb[0:64, qslot * BLK + c0 : (qslot + 1) * BLK],
                        start=True,
                        stop=True,
                    )
                    ex = expool.tile([128, BLK], BF16, name=f"ex_{p}_{j}", tag="ex")
                    if diag:
                        nc.scalar.activation(
                            ex[:, c0:], st_ps[:, c0:], FEXP, bias=0.0
                        )
                        nc.gpsimd.tensor_tensor(
                            ex[:, j * 128 : (j + 1) * 128],
                            ex[:, j * 128 : (j + 1) * 128],
                            triu_sb,
                            mybir.AluOpType.mult,
                        )
                    else:
                        nc.scalar.activation(
                            ex, st_ps, FEXP, bias=bias2_sb[:, p : p + 1]
                        )
                    if av_ps[qslot] is None:
                        av_ps[qslot] = avps_pool.tile(
                            [H + 2, BLK], F32, name=f"av_{qslot}", tag="av"
                        )
                    t = kslot * 4 + j
                    nc.tensor.matmul(
                        av_ps[qslot][:, c0:],
                        vaug_sb[:, t * VA : (t + 1) * VA],
                        ex[:, c0:],
                        start=(first and j == 0),
                        stop=(last and j == 3),
                        skip_group_check=True,
                    )

            def finalize(qs):
                oT_sb = finpool.tile([H + 2, BLK], F32R, name=f"oT_{qs}", tag="oT")
                nc.vector.tensor_copy(oT_sb, av_ps[qs])
                o4_sb = finpool.tile([128, 4 * H], F32, name=f"o4_{qs}", tag="o4")
                for t in range(4):
                    # qs=1 is the kernel tail: double-buffer its transposes
                    # across the vt and (idle by now) q PSUM slots so each
                    # iteration's vector reads overlap the next transpose
                    if qs == 1 and t % 2 == 1:
                        tr_ps = qps_pool.tile(
                            [128, H + 2], F32R, name=f"tr_{qs}_{t}", tag="q"
                        )
                    else:
                        tr_ps = vtps_pool.tile(
                            [128, H + 2], F32R, name=f"tr_{qs}_{t}", tag="vt"
                        )
                    nc.tensor.transpose(
                        tr_ps,
                        oT_sb[:, t * 128 : (t + 1) * 128],
                        idmat_sb[0 : H + 2, 0 : H + 2],
                    )
                    rden = finpool.tile([128, 1], F32, name=f"rd_{qs}_{t}", tag="rd")
                    nc.vector.reciprocal(rden, tr_ps[:, H : H + 1].bitcast(F32))
                    nc.vector.tensor_scalar_mul(
                        o4_sb[:, t * H : (t + 1) * H], tr_ps[:, 0:H].bitcast(F32), rden
                    )
                nc.sync.dma_start(
                    out=out[:, qs * 4 * H : (qs + 1) * 4 * H], in_=o4_sb
                )

            def do_pair(p):
                for j in range(4):
                    do_strip(p, j)

            proj_slot(0)
            do_pair(0)
            proj_slot(1)
            do_pair(1)
            finalize(0)
            proj_slot(2)
            do_pair(2)
            do_pair(3)  # needs only kvT slot1 + qT slot1: run mid-kernel
            proj_slot(3)
            # interleave the last two pairs at strip granularity so the
            # PE always has a matmul ready while the other strip's exp
            # runs; pair 4 is diagonal (narrowed exp) so the tail is
            # lighter on ScalarE than the 3-pair variant
            for j in range(4):
                do_strip(4, j)
                do_strip(5, j)
            finalize(1)

    nc.compile()
    return nc


_NC_CACHE = None
RUN_KWARGS = {}  # test harness may set {"trace": True}
LAST_RESULTS = None  # BassKernelResults of the most recent run


def kernel(x, Wq, Wk, Wv):
    global _NC_CACHE, LAST_RESULTS
    x = np.asarray(x, dtype=np.float32)
    Wq = np.asarray(Wq, dtype=np.float32)
    Wk = np.asarray(Wk, dtype=np.float32)
    Wv = np.asarray(Wv, dtype=np.float32)
    bf = ml_dtypes.bfloat16

    def to_sb(w):  # [E, h] -> [128, NE*h] with e-tiles side by side
        h = w.shape[1]
        return np.ascontiguousarray(
            w.reshape(NE, 128, h).transpose(1, 0, 2).reshape(128, NE * h)
        )

    wq_s = to_sb(Wq / np.float32(E**0.5))
    wkv = to_sb(np.concatenate([Wk, Wv], axis=1))
    wqkv = np.ascontiguousarray(np.concatenate([wkv, wq_s], axis=1)).astype(bf)
    triu = np.triu(np.ones((128, 128), dtype=np.float32)).astype(bf)
    ones2 = np.ones((128, 32), dtype=bf)
    idmat = np.eye(128, dtype=np.float32)
    idmatb = idmat.astype(bf)

    in_maps = []
    for core in range(NCORES):
        b, shard = divmod(core, 2)
        perm = PERM[shard]
        xtf = x[b].T.astype(bf)  # [E, S]
        xt2 = np.concatenate([xtf[:, p * BLK : (p + 1) * BLK] for p in perm], axis=1)
        # pack to [128, slot(4) x etile(8) x col(512)]: any e-range of a
        # slot is contiguous per partition (>=1KB DMA descriptors)
        xt = np.ascontiguousarray(
            xt2.reshape(NE, 128, 4, BLK)
            .transpose(1, 2, 0, 3)
            .reshape(128, NE * S)
        )
        qb = QBLOCKS[shard]
        bias2 = np.zeros((128, 8), dtype=np.float32)
        bias2[:, 6] = NEG
        for p, (qslot, kslot) in enumerate(PAIRS):
            if perm[kslot] > qb[qslot]:  # key block entirely in the future
                bias2[:, p] = NEG
        in_maps.append(
            dict(
                xt=xt,
                wqkv=wqkv,
                bias2=bias2,
                triu=triu,
                idmat=idmat,
                idmatb=idmatb,
                ones2=ones2,
            )
        )

    if _NC_CACHE is None:
        _NC_CACHE = _build()
    res = run_bass_kernel_spmd(
        _NC_CACHE, in_maps, core_ids=list(range(NCORES)), **RUN_KWARGS
    )
    LAST_RESULTS = res

    out = np.empty((B, S, H), dtype=np.float32)
    for core in range(NCORES):
        b, shard = divmod(core, 2)
        # unpack [128, (qs t h)] -> rows qs*512 + t*128 + p
        o = (
            res.results[core]["out"]
            .reshape(128, 8, H)
            .transpose(1, 0, 2)
            .reshape(2, BLK, H)
        )
        for qs, blk in enumerate(QBLOCKS[shard]):
            out[b, blk * BLK : (blk + 1) * BLK, :] = o[qs]
    return out
